# revision 23
# baseline (speedup 1.0000x reference)
"""BlockwiseDense Trainium2 kernel v2 (8 NeuronCores, sharded over out_blocks).

Math (per reference):
    w = rram_quantize(relu(cores))          # snap to 256 log-spaced levels
    y[b,i,j,k] = sum_l w[i,j,k,l] * x[b,j,l]

v2 design (vs baseline): weights upload as fp16 (halves weight DMA);
the decode + output-affine algebra is restructured so each engine does
one cheap pass:

    levels[m] = A - B*r^m,  r = exp(-tau/255)
    w         = A - B*r^n  = g_min - B*(r^n - 1)
    y         = g_min*S[b,j] - B * sum_l (r^n - 1) x[b,j,l],  S = sum_l x

  ACT   : t = Ln((A-c)/B)                  (fp16 out; one pass, all elems)
  DVE   : n = sat_u16(rne(t*MULT + C0))    (16-bit in/out -> 2x rate)
  split : v = r^n - 1 via
            ACT Exp path ((h,jj)=(0,0)): e=Exp(LN_R*n) -> f16 direct
            DVE custom poly (rest):      deg-4 Horner in n -> f16 (exact ints)
  PE    : z += v @ x  plus a constant-rhs matmul that injects
          -(g_min/B)*S into the same PSUM columns  =>  y = -B * PSUM
  evict : pure scale+cast on GpSimd/DVE, then DMA out.

Sharding: core c takes out_blocks i in {2c, 2c+1}; x replicated.
Host pre-transposes x -> x^T fp16 and cores -> (i,jp,p,h,jj,k) fp16.
"""

import numpy as np

import concourse.bacc as bacc
import concourse.mybir as mybir
from concourse.tile import TileContext
from concourse.bass_utils import run_bass_kernel_spmd

# ---- problem constants (hardcoded per contract) ----
BATCH = 128
IN_BLOCKS = 16
OUT_BLOCKS = 16
NB = 256
N_CORES = 8
I_PER_CORE = OUT_BLOCKS // N_CORES  # 2
JP = IN_BLOCKS // 2  # 8 j-pairs

TAU, G_INF, G_MIN, L = 0.75, 2.0, 0.001, 256
B_SCALE = (G_INF - G_MIN) / (1.0 - float(np.exp(-TAU)))
A_OFF = G_MIN + B_SCALE
MULT = -(L - 1) / TAU
LN_R = -TAU / (L - 1)
_r = float(np.exp(LN_R))
DELTA = float(np.log((1 + _r) / 2) / LN_R)
C0 = 0.5 - DELTA  # rne(s) == floor(s+0.5): n = floor(t + 1 - delta)
CMM = -G_MIN / B_SCALE  # const-matmul rhs value: injects -(g_min/B)*S

# deg-4 weighted-LS fit of r^n - 1 = n*(P1 + n*(P2 + n*(P3 + n*P4))),
# n in [0,255]; |err| < 1.2e-5 everywhere, < 4.1e-6 on the occupied range
P1, P2, P3, P4 = -2.9411193e-03, 4.3176460e-06, -4.1047836e-09, 2.2873519e-12

F32 = mybir.dt.float32
F16 = mybir.dt.float16
U8 = mybir.dt.uint8

# tuning knobs
# ACT-Exp path handles (h,jj) = (0,0) for both i (2 of 8 blocks per jp);
# DVE poly handles the rest. Exp emits e = r^n in fp16 directly; the "-1"
# of v = r^n - 1 is injected by an exact -1.0 const-matmul on that (h,jj).
A_HJJ = (0, 0)

_CACHE = {}


def _register_expm1_poly():
    """Custom DVE op: out = in0*(s0 + in0*(s1 + in0*(imm2 + in0*C3))).
    C3 (P4) is spilled to Src1 per the custom-DVE convention. Registered
    once at import; the per-NEFF DVE table is generated at compile."""
    from concourse import dve_ops
    from concourse.dve_spec import Spec, Src0, C0 as S0, C1 as S1, C2 as S2, C3 as S3
    from concourse.dve_spec import lower, _spill_c3_to_src1, _has_src1
    from concourse.dve_uop import DveOpSpec

    name = "EXPM1_POLY_ANT"
    for op in dve_ops.OPS:
        if op.name == name:
            return op
    body = Src0 * (S0 + Src0 * (S1 + Src0 * (S2 + Src0 * S3)))
    spec = Spec(
        body=_spill_c3_to_src1(body),
        reference=lambda in0, in1, s0, s1, imm2: in0
        * (s0 + in0 * (s1 + in0 * (imm2 + in0 * in1))),
    )
    op = dve_ops.DveOp(name, spec, subdim=False, uops_sha={})
    dve_ops.OPS.append(op)
    dve_ops.CUSTOM_DVE_SPECS[name] = spec
    dve_ops._SUB_OPCODE_FOR_NAME[name] = (
        dve_ops._CUSTOM_DVE_ROW_BASE + len(dve_ops.OPS) - 1
    )
    opcode = dve_ops.get_dve_sub_opcode(name)
    for ver in ("v3", "v4"):
        s = DveOpSpec(
            name=name, opcode=opcode, uops=lower(spec, ver=ver),
            rd1_en=_has_src1(spec),
        )
        op.uops_sha[ver] = s.sha(ver)
    return op


EXPM1_POLY = _register_expm1_poly()


class _ForceCombinedLnExpTables:
    """Resolve Ln and Exp to the one table set containing both
    (natural_log_exp_and_others) so the act-table never ping-pongs."""

    def __enter__(self):
        self._orig = bacc.get_activation_tables
        Ln = mybir.ActivationFunctionType.Ln
        Exp = mybir.ActivationFunctionType.Exp

        def patched(arch):
            tabs = self._orig(arch)
            out = {}
            for name, fns in tabs.items():
                if name != "natural_log_exp_and_others" and (Ln in fns or Exp in fns):
                    fns = fns - {Ln, Exp}
                out[name] = fns
            return out

        bacc.get_activation_tables = patched
        return self

    def __exit__(self, *exc):
        bacc.get_activation_tables = self._orig


def _build():
    nc = bacc.Bacc(trn_type="TRN2")
    P = 128
    FD = I_PER_CORE * 2 * 2 * NB  # 2048 w-elems per partition per jp

    # host layouts (per core), partition dim first so whole-tensor DMAs have
    # big contiguous per-partition rows (8KB / 4KB descriptors):
    #   xt: (128, JP, 4, BATCH) f16   -- x^T as (p, jp, c, b); chunk c = 2*jj+h
    #   wt: (JP, 128, I, 2, 2, NB) f16 -- cores^T as (jp, p, i, h, jj, k)
    #   y:  (BATCH, JP, 2, I, NB) f16 -- (b, jp, jj, i, k)
    xt_d = nc.dram_tensor("xt", [P, JP, 4, BATCH], F16, kind="ExternalInput")
    wt_d = nc.dram_tensor(
        "wt", [JP, P, I_PER_CORE, 2, 2, NB], F16, kind="ExternalInput"
    )
    y_d = nc.dram_tensor(
        "y", [BATCH, JP, 2, I_PER_CORE, NB], F16, kind="ExternalOutput"
    )

    with TileContext(nc) as tc:
        with (
            tc.tile_pool(name="singles", bufs=1) as singles,
            tc.tile_pool(name="wraw", bufs=8) as wpool,
            tc.tile_pool(name="tq", bufs=3) as tpool,
            tc.tile_pool(name="nq", bufs=3) as npool,
            tc.tile_pool(name="veq", bufs=3) as vepool,
            tc.tile_pool(name="vq", bufs=3) as vpool,
            tc.tile_pool(name="yout", bufs=4) as ypool,
            tc.tile_pool(name="yps", bufs=4, space="PSUM") as yps,
        ):
            # w_jp0 DMA first-thing from gpsimd: lands ~2us before the
            # sync-queue path would deliver it
            wraw0 = wpool.tile([P, I_PER_CORE, 2, 2, NB], F16, tag="wraw")
            nc.gpsimd.dma_start(out=wraw0[:], in_=wt_d[0])

            bias_ln = singles.tile([P, 1], F32)
            nc.vector.memset(bias_ln[:], A_OFF / B_SCALE)
            p4t = singles.tile([P, 1], F32)
            nc.vector.memset(p4t[:], P4)
            cmm = singles.tile([P, 2 * NB], F16)
            nc.vector.memset(cmm[:], CMM)
            cm1 = singles.tile([P, 2 * NB], F16)
            nc.vector.memset(cm1[:], -1.0)
            # pull the Ln/Exp ACT table load off the critical path: a dummy
            # 1-elem activation whose deps are ready at t~0
            dummy = singles.tile([P, 1], F32)
            nc.scalar.activation(
                dummy[:], bias_ln[:], mybir.ActivationFunctionType.Ln,
                bias=0.0, scale=1.0,
            )
            # w1..w7 issued back-to-back on sync BEFORE x and the y-outs:
            # later dma_starts in the sync FIFO wait on their inputs, so
            # anything ahead of the w stream head-of-line-blocks it.
            wraws = [wraw0]
            for jp in range(1, JP):
                wr = wpool.tile([P, I_PER_CORE, 2, 2, NB], F16, tag="wraw")
                nc.sync.dma_start(out=wr[:], in_=wt_d[jp])
                wraws.append(wr)
            # full x^T resident in SBUF (8KB/partition), one DMA from sync
            xt_sb = singles.tile([P, JP, 4, BATCH], F16)
            nc.sync.dma_start(out=xt_sb[:], in_=xt_d[:, :])

            flat = "p a b c k -> p (a b c k)"
            for jp in range(JP):
                wraw = wraws[jp]
                wflat = wraw[:].rearrange(flat)

                # jp0 runs the quantize chain per-i to fill the pipeline
                tp = tpool.tile([P, FD], F32, tag="tp")
                nu = npool.tile([P, FD], U8, tag="nu")
                ve = vepool.tile([P, I_PER_CORE, NB], F16, tag="ve")
                vp = vpool.tile([P, I_PER_CORE, 3 * NB], F16, tag="vp")
                nu_r = nu[:].rearrange("p (i q) -> p i q", i=I_PER_CORE)
                igroups = (
                    [(0, 1), (1, 2)] if jp == 0 else [(0, I_PER_CORE)]
                )
                for i0, i1 in igroups:
                    lo, hi = i0 * 1024, i1 * 1024
                    # ---- quantize index: Ln -> u8(rne+sat) ----
                    nc.scalar.activation(
                        tp[:, lo:hi], wflat[:, lo:hi],
                        mybir.ActivationFunctionType.Ln,
                        bias=bias_ln[:, 0:1], scale=-1.0 / B_SCALE,
                    )
                    nc.gpsimd.tensor_scalar(
                        nu[:, lo:hi], tp[:, lo:hi], MULT, C0,
                        mybir.AluOpType.mult, mybir.AluOpType.add,
                    )
                    # ---- decode: e = r^n (f16) on ACT for (h,jj)=(0,0);
                    #      v = r^n - 1 (f16) on DVE poly for the rest ----
                    nc.scalar.activation(
                        ve[:, i0:i1], nu_r[:, i0:i1, 0:NB],
                        mybir.ActivationFunctionType.Exp,
                        bias=0.0, scale=LN_R,
                    )
                    nc.vector._custom_dve(
                        EXPM1_POLY, out=vp[:, i0:i1],
                        in0=nu_r[:, i0:i1, NB : 4 * NB],
                        in1=p4t[:, 0:1], s0=P1, s1=P2, imm2=P3,
                    )

                def vslice(i, h, jj):
                    if (h, jj) == A_HJJ:
                        return ve[:, i, :]
                    off = h * 512 + jj * NB - NB
                    return vp[:, i, off : off + NB]

                # ---- matmuls: z cols = jj*512 + i*256 + k ----
                # PSUM start=True zeroes a whole 2KB zero-region, so the
                # N=512 const-mm (spanning exactly one region) must be the
                # sole start; every v-mm accumulates. The A_HJJ (h,jj) gets
                # an extra exact -1.0 const-mm (its rhs is e, not e-1).
                yp = yps.tile([P, 2 * I_PER_CORE * NB], F32, tag="yp")
                for jj in range(2):
                    for h in range(2):
                        lhsT = xt_sb[:, jp, 2 * jj + h, :]
                        nc.tensor.matmul(
                            yp[:, jj * 512 : jj * 512 + 512],
                            lhsT, cmm[:, 0:512],
                            start=(h == 0), stop=False,
                        )
                        if (h, jj) == A_HJJ:
                            nc.tensor.matmul(
                                yp[:, jj * 512 : jj * 512 + 512],
                                lhsT, cm1[:, 0:512],
                                start=False, stop=False,
                            )
                        for i in range(I_PER_CORE):
                            dst = jj * 512 + i * NB
                            nc.tensor.matmul(
                                yp[:, dst : dst + NB],
                                lhsT, vslice(i, h, jj),
                                start=False, stop=(h == 1),
                            )

                # ---- evict: y = -B * PSUM, cast f16; store ----
                ysb = ypool.tile([P, 2 * I_PER_CORE * NB], F16, tag="ysb")
                if jp % 4 == 3:  # keep DVE and ACT evict loads balanced
                    nc.scalar.activation(
                        ysb[:], yp[:], mybir.ActivationFunctionType.Copy,
                        bias=0.0, scale=-B_SCALE,
                    )
                else:
                    nc.vector.tensor_scalar(
                        ysb[:], yp[:], -B_SCALE, None, mybir.AluOpType.mult
                    )
                nc.sync.dma_start(out=y_d[:, jp], in_=ysb[:])

    with _ForceCombinedLnExpTables():
        nc.compile()
    return nc


def _get_nc():
    if "nc" not in _CACHE:
        _CACHE["nc"] = _build()
    return _CACHE["nc"]


def kernel(x: np.ndarray, cores: np.ndarray, _trace=False, _trace_kwargs=None):
    x = np.asarray(x, dtype=np.float32)
    cores = np.asarray(cores, dtype=np.float32)

    # x^T in fp16, laid out (p, jp, c, b), c = 2*jj + h
    xt = np.ascontiguousarray(
        x.T.reshape(JP, 4, 128, BATCH).transpose(2, 0, 1, 3).astype(np.float16)
    )
    # cores^T: (i,j,l,k) -> (jp, p, i, h, jj, k), fp16
    wt = (
        cores.transpose(0, 1, 3, 2)
        .reshape(OUT_BLOCKS, JP, 2, 2, 128, NB)
        .transpose(1, 4, 0, 3, 2, 5)
        .astype(np.float16)
    )
    in_maps = [
        {
            "xt": xt,
            "wt": np.ascontiguousarray(
                wt[:, :, c * I_PER_CORE : (c + 1) * I_PER_CORE]
            ),
        }
        for c in range(N_CORES)
    ]

    nc = _get_nc()
    kw = {}
    if _trace:
        kw = dict(trace=True, **(_trace_kwargs or {}))
    out = run_bass_kernel_spmd(nc, in_maps, core_ids=list(range(N_CORES)), **kw)
    if _trace:
        _CACHE["last_result"] = out
    y = np.concatenate(
        [
            r["y"]  # (b, jp, jj, i, k)
            .astype(np.float32)
            .transpose(0, 3, 1, 2, 4)  # (b, i, jp, jj, k)
            .reshape(BATCH, I_PER_CORE, IN_BLOCKS, NB)
            for r in out.results
        ],
        axis=1,
    )
    return y


# revision 26
# speedup vs baseline: 1.0112x; 1.0112x over previous
"""BlockwiseDense Trainium2 kernel v2 (8 NeuronCores, sharded over out_blocks).

Math (per reference):
    w = rram_quantize(relu(cores))          # snap to 256 log-spaced levels
    y[b,i,j,k] = sum_l w[i,j,k,l] * x[b,j,l]

v2 design (vs baseline): weights upload as fp16 (halves weight DMA);
the decode + output-affine algebra is restructured so each engine does
one cheap pass:

    levels[m] = A - B*r^m,  r = exp(-tau/255)
    w         = A - B*r^n  = g_min - B*(r^n - 1)
    y         = g_min*S[b,j] - B * sum_l (r^n - 1) x[b,j,l],  S = sum_l x

  ACT   : t = Ln((A-c)/B)                  (fp16 out; one pass, all elems)
  DVE   : n = sat_u16(rne(t*MULT + C0))    (16-bit in/out -> 2x rate)
  split : v = r^n - 1 via
            ACT Exp path ((h,jj)=(0,0)): e=Exp(LN_R*n) -> f16 direct
            DVE custom poly (rest):      deg-4 Horner in n -> f16 (exact ints)
  PE    : z += v @ x  plus a constant-rhs matmul that injects
          -(g_min/B)*S into the same PSUM columns  =>  y = -B * PSUM
  evict : pure scale+cast on GpSimd/DVE, then DMA out.

Sharding: core c takes out_blocks i in {2c, 2c+1}; x replicated.
Host pre-transposes x -> x^T fp16 and cores -> (i,jp,p,h,jj,k) fp16.
"""

import numpy as np

import concourse.bacc as bacc
import concourse.mybir as mybir
from concourse.tile import TileContext
from concourse.bass_utils import run_bass_kernel_spmd

# ---- problem constants (hardcoded per contract) ----
BATCH = 128
IN_BLOCKS = 16
OUT_BLOCKS = 16
NB = 256
N_CORES = 8
I_PER_CORE = OUT_BLOCKS // N_CORES  # 2
JP = IN_BLOCKS // 2  # 8 j-pairs

TAU, G_INF, G_MIN, L = 0.75, 2.0, 0.001, 256
B_SCALE = (G_INF - G_MIN) / (1.0 - float(np.exp(-TAU)))
A_OFF = G_MIN + B_SCALE
MULT = -(L - 1) / TAU
LN_R = -TAU / (L - 1)
_r = float(np.exp(LN_R))
DELTA = float(np.log((1 + _r) / 2) / LN_R)
C0 = 0.5 - DELTA  # rne(s) == floor(s+0.5): n = floor(t + 1 - delta)
CMM = -G_MIN / B_SCALE  # const-matmul rhs value: injects -(g_min/B)*S

# deg-4 weighted-LS fit of r^n - 1 = n*(P1 + n*(P2 + n*(P3 + n*P4))),
# n in [0,255]; |err| < 1.2e-5 everywhere, < 4.1e-6 on the occupied range
P1, P2, P3, P4 = -2.9411193e-03, 4.3176460e-06, -4.1047836e-09, 2.2873519e-12

F32 = mybir.dt.float32
F16 = mybir.dt.float16
U8 = mybir.dt.uint8

# tuning knobs
# ACT-Exp path handles (h,jj) = (0,0) for both i (2 of 8 blocks per jp);
# DVE poly handles the rest. Exp emits e = r^n in fp16 directly; the "-1"
# of v = r^n - 1 is injected by an exact -1.0 const-matmul on that (h,jj).
A_HJJ = (0, 0)

_CACHE = {}


def _register_expm1_poly():
    """Custom DVE op: out = in0*(s0 + in0*(s1 + in0*(imm2 + in0*C3))).
    C3 (P4) is spilled to Src1 per the custom-DVE convention. Registered
    once at import; the per-NEFF DVE table is generated at compile."""
    from concourse import dve_ops
    from concourse.dve_spec import Spec, Src0, C0 as S0, C1 as S1, C2 as S2, C3 as S3
    from concourse.dve_spec import lower, _spill_c3_to_src1, _has_src1
    from concourse.dve_uop import DveOpSpec

    name = "EXPM1_POLY_ANT"
    for op in dve_ops.OPS:
        if op.name == name:
            return op
    body = Src0 * (S0 + Src0 * (S1 + Src0 * (S2 + Src0 * S3)))
    spec = Spec(
        body=_spill_c3_to_src1(body),
        reference=lambda in0, in1, s0, s1, imm2: in0
        * (s0 + in0 * (s1 + in0 * (imm2 + in0 * in1))),
    )
    op = dve_ops.DveOp(name, spec, subdim=False, uops_sha={})
    dve_ops.OPS.append(op)
    dve_ops.CUSTOM_DVE_SPECS[name] = spec
    dve_ops._SUB_OPCODE_FOR_NAME[name] = (
        dve_ops._CUSTOM_DVE_ROW_BASE + len(dve_ops.OPS) - 1
    )
    opcode = dve_ops.get_dve_sub_opcode(name)
    for ver in ("v3", "v4"):
        s = DveOpSpec(
            name=name, opcode=opcode, uops=lower(spec, ver=ver),
            rd1_en=_has_src1(spec),
        )
        op.uops_sha[ver] = s.sha(ver)
    return op


EXPM1_POLY = _register_expm1_poly()


class _ForceCombinedLnExpTables:
    """Resolve Ln and Exp to the one table set containing both
    (natural_log_exp_and_others) so the act-table never ping-pongs."""

    def __enter__(self):
        self._orig = bacc.get_activation_tables
        Ln = mybir.ActivationFunctionType.Ln
        Exp = mybir.ActivationFunctionType.Exp

        def patched(arch):
            tabs = self._orig(arch)
            out = {}
            for name, fns in tabs.items():
                if name != "natural_log_exp_and_others" and (Ln in fns or Exp in fns):
                    fns = fns - {Ln, Exp}
                out[name] = fns
            return out

        bacc.get_activation_tables = patched
        return self

    def __exit__(self, *exc):
        bacc.get_activation_tables = self._orig


def _build():
    nc = bacc.Bacc(trn_type="TRN2")
    P = 128
    FD = I_PER_CORE * 2 * 2 * NB  # 2048 w-elems per partition per jp

    # host layouts (per core), partition dim first so whole-tensor DMAs have
    # big contiguous per-partition rows (8KB / 4KB descriptors):
    #   xt: (128, JP, 4, BATCH) f16   -- x^T as (p, jp, c, b); chunk c = 2*jj+h
    #   wt: (JP, 128, I, 2, 2, NB) f16 -- cores^T as (jp, p, i, h, jj, k)
    #   y:  (BATCH, JP, 2, I, NB) f16 -- (b, jp, jj, i, k)
    xt_d = nc.dram_tensor("xt", [P, JP, 4, BATCH], F16, kind="ExternalInput")
    wt_d = nc.dram_tensor(
        "wt", [JP, P, I_PER_CORE, 2, 2, NB], F16, kind="ExternalInput"
    )
    y_d = nc.dram_tensor(
        "y", [BATCH, JP, 2, I_PER_CORE, NB], F16, kind="ExternalOutput"
    )

    with TileContext(nc) as tc:
        with (
            tc.tile_pool(name="singles", bufs=1) as singles,
            tc.tile_pool(name="wraw", bufs=8) as wpool,
            tc.tile_pool(name="tq", bufs=3) as tpool,
            tc.tile_pool(name="nq", bufs=3) as npool,
            tc.tile_pool(name="veq", bufs=3) as vepool,
            tc.tile_pool(name="vq", bufs=3) as vpool,
            tc.tile_pool(name="yout", bufs=4) as ypool,
            tc.tile_pool(name="yps", bufs=4, space="PSUM") as yps,
        ):
            # w_jp0 DMA first-thing from gpsimd: lands ~2us before the
            # sync-queue path would deliver it
            wraw0 = wpool.tile([P, I_PER_CORE, 2, 2, NB], F16, tag="wraw")
            nc.gpsimd.dma_start(out=wraw0[:], in_=wt_d[0])

            bias_ln = singles.tile([P, 1], F32)
            nc.vector.memset(bias_ln[:], A_OFF / B_SCALE)
            p4t = singles.tile([P, 1], F32)
            nc.vector.memset(p4t[:], P4)
            cmm = singles.tile([P, 2 * NB], F16)
            nc.vector.memset(cmm[:], CMM)
            cm1 = singles.tile([P, 2 * NB], F16)
            nc.vector.memset(cm1[:], -1.0)
            # pull the Ln/Exp ACT table load off the critical path: a dummy
            # 1-elem activation whose deps are ready at t~0
            dummy = singles.tile([P, 1], F32)
            nc.scalar.activation(
                dummy[:], bias_ln[:], mybir.ActivationFunctionType.Ln,
                bias=0.0, scale=1.0,
            )
            # w1..w7 issued back-to-back on sync BEFORE x and the y-outs:
            # later dma_starts in the sync FIFO wait on their inputs, so
            # anything ahead of the w stream head-of-line-blocks it.
            # the 16 DMA queues drain in issue order: w0, w1, then x (x is
            # only needed by the first matmuls ~13us in), then w2..w7
            wraws = [wraw0]
            xt_sb = singles.tile([P, JP, 4, BATCH], F16)
            for jp in range(1, JP):
                wr = wpool.tile([P, I_PER_CORE, 2, 2, NB], F16, tag="wraw")
                nc.sync.dma_start(out=wr[:], in_=wt_d[jp])
                wraws.append(wr)
                if jp == 1:
                    nc.sync.dma_start(out=xt_sb[:], in_=xt_d[:, :])

            flat = "p a b c k -> p (a b c k)"
            for jp in range(JP):
                wraw = wraws[jp]
                wflat = wraw[:].rearrange(flat)

                # jp0 runs the quantize chain per-i to fill the pipeline
                tp = tpool.tile([P, FD], F32, tag="tp")
                nu = npool.tile([P, FD], U8, tag="nu")
                ve = vepool.tile([P, I_PER_CORE, NB], F16, tag="ve")
                vp = vpool.tile([P, I_PER_CORE, 3 * NB], F16, tag="vp")
                nu_r = nu[:].rearrange("p (i q) -> p i q", i=I_PER_CORE)
                igroups = (
                    [(0, 1), (1, 2)] if jp == 0 else [(0, I_PER_CORE)]
                )
                for i0, i1 in igroups:
                    lo, hi = i0 * 1024, i1 * 1024
                    # ---- quantize index: Ln -> u8(rne+sat) ----
                    nc.scalar.activation(
                        tp[:, lo:hi], wflat[:, lo:hi],
                        mybir.ActivationFunctionType.Ln,
                        bias=bias_ln[:, 0:1], scale=-1.0 / B_SCALE,
                    )
                    # rounds for the last jps go on DVE: the scheduler's
                    # gpsimd cost model is pessimistic, which otherwise makes
                    # it queue evicts ahead of the final polys on DVE
                    reng = nc.vector if jp >= 6 else nc.gpsimd
                    reng.tensor_scalar(
                        nu[:, lo:hi], tp[:, lo:hi], MULT, C0,
                        mybir.AluOpType.mult, mybir.AluOpType.add,
                    )
                    # ---- decode: e = r^n (f16) on ACT for (h,jj)=(0,0);
                    #      v = r^n - 1 (f16) on DVE poly for the rest ----
                    nc.scalar.activation(
                        ve[:, i0:i1], nu_r[:, i0:i1, 0:NB],
                        mybir.ActivationFunctionType.Exp,
                        bias=0.0, scale=LN_R,
                    )
                    nc.vector._custom_dve(
                        EXPM1_POLY, out=vp[:, i0:i1],
                        in0=nu_r[:, i0:i1, NB : 4 * NB],
                        in1=p4t[:, 0:1], s0=P1, s1=P2, imm2=P3,
                    )

                def vslice(i, h, jj):
                    if (h, jj) == A_HJJ:
                        return ve[:, i, :]
                    off = h * 512 + jj * NB - NB
                    return vp[:, i, off : off + NB]

                # ---- matmuls: z cols = jj*512 + i*256 + k ----
                # PSUM start=True zeroes a whole 2KB zero-region, so the
                # N=512 const-mm (spanning exactly one region) must be the
                # sole start; every v-mm accumulates. The A_HJJ (h,jj) gets
                # an extra exact -1.0 const-mm (its rhs is e, not e-1).
                yp = yps.tile([P, 2 * I_PER_CORE * NB], F32, tag="yp")
                for jj in range(2):
                    for h in range(2):
                        lhsT = xt_sb[:, jp, 2 * jj + h, :]
                        nc.tensor.matmul(
                            yp[:, jj * 512 : jj * 512 + 512],
                            lhsT, cmm[:, 0:512],
                            start=(h == 0), stop=False,
                        )
                        if (h, jj) == A_HJJ:
                            nc.tensor.matmul(
                                yp[:, jj * 512 : jj * 512 + 512],
                                lhsT, cm1[:, 0:512],
                                start=False, stop=False,
                            )
                        for i in range(I_PER_CORE):
                            dst = jj * 512 + i * NB
                            nc.tensor.matmul(
                                yp[:, dst : dst + NB],
                                lhsT, vslice(i, h, jj),
                                start=False, stop=(h == 1),
                            )

                # ---- evict: y = -B * PSUM, cast f16; store ----
                ysb = ypool.tile([P, 2 * I_PER_CORE * NB], F16, tag="ysb")
                if jp == 4:  # keep DVE and ACT evict loads balanced
                    nc.scalar.activation(
                        ysb[:], yp[:], mybir.ActivationFunctionType.Copy,
                        bias=0.0, scale=-B_SCALE,
                    )
                else:
                    nc.vector.tensor_scalar(
                        ysb[:], yp[:], -B_SCALE, None, mybir.AluOpType.mult
                    )
                nc.sync.dma_start(out=y_d[:, jp], in_=ysb[:])

    with _ForceCombinedLnExpTables():
        nc.compile()
    return nc


def _get_nc():
    if "nc" not in _CACHE:
        _CACHE["nc"] = _build()
    return _CACHE["nc"]


def kernel(x: np.ndarray, cores: np.ndarray, _trace=False, _trace_kwargs=None):
    x = np.asarray(x, dtype=np.float32)
    cores = np.asarray(cores, dtype=np.float32)

    # x^T in fp16, laid out (p, jp, c, b), c = 2*jj + h
    xt = np.ascontiguousarray(
        x.T.reshape(JP, 4, 128, BATCH).transpose(2, 0, 1, 3).astype(np.float16)
    )
    # cores^T: (i,j,l,k) -> (jp, p, i, h, jj, k), fp16
    wt = (
        cores.transpose(0, 1, 3, 2)
        .reshape(OUT_BLOCKS, JP, 2, 2, 128, NB)
        .transpose(1, 4, 0, 3, 2, 5)
        .astype(np.float16)
    )
    in_maps = [
        {
            "xt": xt,
            "wt": np.ascontiguousarray(
                wt[:, :, c * I_PER_CORE : (c + 1) * I_PER_CORE]
            ),
        }
        for c in range(N_CORES)
    ]

    nc = _get_nc()
    kw = {}
    if _trace:
        kw = dict(trace=True, **(_trace_kwargs or {}))
    out = run_bass_kernel_spmd(nc, in_maps, core_ids=list(range(N_CORES)), **kw)
    if _trace:
        _CACHE["last_result"] = out
    y = np.concatenate(
        [
            r["y"]  # (b, jp, jj, i, k)
            .astype(np.float32)
            .transpose(0, 3, 1, 2, 4)  # (b, i, jp, jj, k)
            .reshape(BATCH, I_PER_CORE, IN_BLOCKS, NB)
            for r in out.results
        ],
        axis=1,
    )
    return y


# revision 29
# speedup vs baseline: 1.0596x; 1.0478x over previous
"""BlockwiseDense Trainium2 kernel v2 (8 NeuronCores, sharded over out_blocks).

Math (per reference):
    w = rram_quantize(relu(cores))          # snap to 256 log-spaced levels
    y[b,i,j,k] = sum_l w[i,j,k,l] * x[b,j,l]

v2 design (vs baseline): weights upload as fp16 (halves weight DMA);
the decode + output-affine algebra is restructured so each engine does
one cheap pass:

    levels[m] = A - B*r^m,  r = exp(-tau/255)
    w         = A - B*r^n  = g_min - B*(r^n - 1)
    y         = g_min*S[b,j] - B * sum_l (r^n - 1) x[b,j,l],  S = sum_l x

  ACT   : t = Ln((A-c)/B)                  (fp16 out; one pass, all elems)
  DVE   : n = sat_u16(rne(t*MULT + C0))    (16-bit in/out -> 2x rate)
  split : v = r^n - 1 via
            ACT Exp path ((h,jj)=(0,0)): e=Exp(LN_R*n) -> f16 direct
            DVE custom poly (rest):      deg-4 Horner in n -> f16 (exact ints)
  PE    : z += v @ x  plus a constant-rhs matmul that injects
          -(g_min/B)*S into the same PSUM columns  =>  y = -B * PSUM
  evict : pure scale+cast on GpSimd/DVE, then DMA out.

Sharding: core c takes out_blocks i in {2c, 2c+1}; x replicated.
Host pre-transposes x -> x^T fp16 and cores -> (i,jp,p,h,jj,k) fp16.
"""

import numpy as np

import concourse.bacc as bacc
import concourse.mybir as mybir
from concourse.tile import TileContext
from concourse.bass_utils import run_bass_kernel_spmd

# ---- problem constants (hardcoded per contract) ----
BATCH = 128
IN_BLOCKS = 16
OUT_BLOCKS = 16
NB = 256
N_CORES = 8
I_PER_CORE = OUT_BLOCKS // N_CORES  # 2
JP = IN_BLOCKS // 2  # 8 j-pairs

TAU, G_INF, G_MIN, L = 0.75, 2.0, 0.001, 256
B_SCALE = (G_INF - G_MIN) / (1.0 - float(np.exp(-TAU)))
A_OFF = G_MIN + B_SCALE
MULT = -(L - 1) / TAU
LN_R = -TAU / (L - 1)
_r = float(np.exp(LN_R))
DELTA = float(np.log((1 + _r) / 2) / LN_R)
C0 = 0.5 - DELTA  # rne(s) == floor(s+0.5): n = floor(t + 1 - delta)
CMM = -G_MIN / B_SCALE  # const-matmul rhs value: injects -(g_min/B)*S

# deg-4 weighted-LS fit of r^n - 1 = n*(P1 + n*(P2 + n*(P3 + n*P4))),
# n in [0,255]; |err| < 1.2e-5 everywhere, < 4.1e-6 on the occupied range
P1, P2, P3, P4 = -2.9411193e-03, 4.3176460e-06, -4.1047836e-09, 2.2873519e-12

F32 = mybir.dt.float32
F16 = mybir.dt.float16
U8 = mybir.dt.uint8

# tuning knobs
# ACT-Exp path handles (h,jj) = (0,0) for both i (2 of 8 blocks per jp);
# DVE poly handles the rest. Exp emits e = r^n in fp16 directly; the "-1"
# of v = r^n - 1 is injected by an exact -1.0 const-matmul on that (h,jj).
A_HJJ = (0, 0)

_CACHE = {}


def _register_expm1_poly():
    """Custom DVE op: out = in0*(s0 + in0*(s1 + in0*(imm2 + in0*C3))).
    C3 (P4) is spilled to Src1 per the custom-DVE convention. Registered
    once at import; the per-NEFF DVE table is generated at compile."""
    from concourse import dve_ops
    from concourse.dve_spec import Spec, Src0, C0 as S0, C1 as S1, C2 as S2, C3 as S3
    from concourse.dve_spec import lower, _spill_c3_to_src1, _has_src1
    from concourse.dve_uop import DveOpSpec

    name = "EXPM1_POLY_ANT"
    for op in dve_ops.OPS:
        if op.name == name:
            return op
    body = Src0 * (S0 + Src0 * (S1 + Src0 * (S2 + Src0 * S3)))
    spec = Spec(
        body=_spill_c3_to_src1(body),
        reference=lambda in0, in1, s0, s1, imm2: in0
        * (s0 + in0 * (s1 + in0 * (imm2 + in0 * in1))),
    )
    op = dve_ops.DveOp(name, spec, subdim=False, uops_sha={})
    dve_ops.OPS.append(op)
    dve_ops.CUSTOM_DVE_SPECS[name] = spec
    dve_ops._SUB_OPCODE_FOR_NAME[name] = (
        dve_ops._CUSTOM_DVE_ROW_BASE + len(dve_ops.OPS) - 1
    )
    opcode = dve_ops.get_dve_sub_opcode(name)
    for ver in ("v3", "v4"):
        s = DveOpSpec(
            name=name, opcode=opcode, uops=lower(spec, ver=ver),
            rd1_en=_has_src1(spec),
        )
        op.uops_sha[ver] = s.sha(ver)
    return op


EXPM1_POLY = _register_expm1_poly()


class _ForceCombinedLnExpTables:
    """Resolve Ln and Exp to the one table set containing both
    (natural_log_exp_and_others) so the act-table never ping-pongs."""

    def __enter__(self):
        self._orig = bacc.get_activation_tables
        Ln = mybir.ActivationFunctionType.Ln
        Exp = mybir.ActivationFunctionType.Exp

        def patched(arch):
            tabs = self._orig(arch)
            out = {}
            for name, fns in tabs.items():
                if name != "natural_log_exp_and_others" and (Ln in fns or Exp in fns):
                    fns = fns - {Ln, Exp}
                out[name] = fns
            return out

        bacc.get_activation_tables = patched
        return self

    def __exit__(self, *exc):
        bacc.get_activation_tables = self._orig


def _build():
    nc = bacc.Bacc(trn_type="TRN2")
    P = 128
    FD = I_PER_CORE * 2 * 2 * NB  # 2048 w-elems per partition per jp

    # host layouts (per core), partition dim first so whole-tensor DMAs have
    # big contiguous per-partition rows (8KB / 4KB descriptors):
    #   xt: (128, JP, 4, BATCH) f16   -- x^T as (p, jp, c, b); chunk c = 2*jj+h
    #   wt: (JP, 128, I, 2, 2, NB) f16 -- cores^T as (jp, p, i, h, jj, k)
    #   y:  (BATCH, JP, 2, I, NB) f16 -- (b, jp, jj, i, k)
    xt_d = nc.dram_tensor("xt", [P, JP, 4, BATCH], F16, kind="ExternalInput")
    wt_d = nc.dram_tensor(
        "wt", [JP, P, I_PER_CORE, 2, 2, NB], F16, kind="ExternalInput"
    )
    y_d = nc.dram_tensor(
        "y", [BATCH, JP, 2, I_PER_CORE, NB], F16, kind="ExternalOutput"
    )

    with TileContext(nc) as tc:
        with (
            tc.tile_pool(name="singles", bufs=1) as singles,
            tc.tile_pool(name="wraw", bufs=8) as wpool,
            tc.tile_pool(name="tq", bufs=3) as tpool,
            tc.tile_pool(name="nq", bufs=3) as npool,
            tc.tile_pool(name="veq", bufs=3) as vepool,
            tc.tile_pool(name="vq", bufs=3) as vpool,
            tc.tile_pool(name="yout", bufs=4) as ypool,
            tc.tile_pool(name="yps", bufs=4, space="PSUM") as yps,
        ):
            # w_jp0 first in the sync HWDGE queue, split per i so the jp0
            # chain can start on the first half
            wraw0 = wpool.tile([P, I_PER_CORE, 2, 2, NB], F16, tag="wraw")
            for i in range(I_PER_CORE):
                nc.sync.dma_start(out=wraw0[:, i], in_=wt_d[0, :, i])

            bias_ln = singles.tile([P, 1], F32)
            nc.vector.memset(bias_ln[:], A_OFF / B_SCALE)
            p4t = singles.tile([P, 1], F32)
            nc.vector.memset(p4t[:], P4)
            cmm = singles.tile([P, 2 * NB], F16)
            nc.vector.memset(cmm[:], CMM)
            cm1 = singles.tile([P, 2 * NB], F16)
            nc.vector.memset(cm1[:], -1.0)
            # pull the Ln/Exp ACT table load off the critical path: a dummy
            # 1-elem activation whose deps are ready at t~0
            dummy = singles.tile([P, 1], F32)
            nc.scalar.activation(
                dummy[:], bias_ln[:], mybir.ActivationFunctionType.Ln,
                bias=0.0, scale=1.0,
            )
            # w1..w7 issued back-to-back on sync BEFORE x and the y-outs:
            # later dma_starts in the sync FIFO wait on their inputs, so
            # anything ahead of the w stream head-of-line-blocks it.
            # the 16 DMA queues drain in issue order: w0, w1, then x (x is
            # only needed by the first matmuls ~13us in), then w2..w7
            wraws = [wraw0]
            xt_sb = singles.tile([P, JP, 4, BATCH], F16)
            for jp in range(1, JP):
                wr = wpool.tile([P, I_PER_CORE, 2, 2, NB], F16, tag="wraw")
                nc.sync.dma_start(out=wr[:], in_=wt_d[jp])
                wraws.append(wr)
                if jp == 1:
                    nc.sync.dma_start(out=xt_sb[:], in_=xt_d[:, :])

            flat = "p a b c k -> p (a b c k)"
            for jp in range(JP):
                wraw = wraws[jp]
                wflat = wraw[:].rearrange(flat)

                # jp0 runs the quantize chain per-i to fill the pipeline
                tp = tpool.tile([P, FD], F32, tag="tp")
                nu = npool.tile([P, FD], U8, tag="nu")
                ve = vepool.tile([P, I_PER_CORE, NB], F16, tag="ve")
                vp = vpool.tile([P, I_PER_CORE, 3 * NB], F16, tag="vp")
                nu_r = nu[:].rearrange("p (i q) -> p i q", i=I_PER_CORE)
                # graded granularity: fine chunks at the head (pipeline fill)
                # and the tail (drain) of the jp loop
                igroups = (
                    [(0, 1), (1, 2)] if jp in (0, 6, 7) else [(0, I_PER_CORE)]
                )
                for i0, i1 in igroups:
                    lo, hi = i0 * 1024, i1 * 1024
                    # ---- quantize index: Ln -> u8(rne+sat) ----
                    nc.scalar.activation(
                        tp[:, lo:hi], wflat[:, lo:hi],
                        mybir.ActivationFunctionType.Ln,
                        bias=bias_ln[:, 0:1], scale=-1.0 / B_SCALE,
                    )
                    # round for the last jp goes on DVE: the scheduler's
                    # gpsimd cost model is pessimistic, which otherwise makes
                    # it queue evicts ahead of the final polys on DVE
                    reng = nc.vector if jp >= 7 else nc.gpsimd
                    reng.tensor_scalar(
                        nu[:, lo:hi], tp[:, lo:hi], MULT, C0,
                        mybir.AluOpType.mult, mybir.AluOpType.add,
                    )
                    # ---- decode: e = r^n (f16) on ACT for (h,jj)=(0,0);
                    #      v = r^n - 1 (f16) on DVE poly for the rest ----
                    nc.scalar.activation(
                        ve[:, i0:i1], nu_r[:, i0:i1, 0:NB],
                        mybir.ActivationFunctionType.Exp,
                        bias=0.0, scale=LN_R,
                    )
                    psplit = (
                        [(NB, 2 * NB), (2 * NB, 4 * NB)]
                        if jp == 7
                        else [(NB, 4 * NB)]
                    )
                    for q0, q1 in psplit:
                        nc.vector._custom_dve(
                            EXPM1_POLY,
                            out=vp[:, i0:i1, q0 - NB : q1 - NB],
                            in0=nu_r[:, i0:i1, q0:q1],
                            in1=p4t[:, 0:1], s0=P1, s1=P2, imm2=P3,
                        )

                def vslice(i, h, jj):
                    if (h, jj) == A_HJJ:
                        return ve[:, i, :]
                    off = h * 512 + jj * NB - NB
                    return vp[:, i, off : off + NB]

                # ---- matmuls: z cols = jj*512 + i*256 + k ----
                # PSUM start=True zeroes a whole 2KB zero-region, so the
                # N=512 const-mm (spanning exactly one region) must be the
                # sole start; every v-mm accumulates. The A_HJJ (h,jj) gets
                # an extra exact -1.0 const-mm (its rhs is e, not e-1).
                yp = yps.tile([P, 2 * I_PER_CORE * NB], F32, tag="yp")
                for jj in range(2):
                    for h in range(2):
                        lhsT = xt_sb[:, jp, 2 * jj + h, :]
                        nc.tensor.matmul(
                            yp[:, jj * 512 : jj * 512 + 512],
                            lhsT, cmm[:, 0:512],
                            start=(h == 0), stop=False,
                        )
                        if (h, jj) == A_HJJ:
                            nc.tensor.matmul(
                                yp[:, jj * 512 : jj * 512 + 512],
                                lhsT, cm1[:, 0:512],
                                start=False, stop=False,
                            )
                        for i in range(I_PER_CORE):
                            dst = jj * 512 + i * NB
                            nc.tensor.matmul(
                                yp[:, dst : dst + NB],
                                lhsT, vslice(i, h, jj),
                                start=False, stop=(h == 1),
                            )

                # ---- evict: y = -B * PSUM, cast f16; store ----
                ysb = ypool.tile([P, 2 * I_PER_CORE * NB], F16, tag="ysb")
                if jp in (3, 6):  # keep DVE and ACT evict loads balanced
                    nc.scalar.activation(
                        ysb[:], yp[:], mybir.ActivationFunctionType.Copy,
                        bias=0.0, scale=-B_SCALE,
                    )
                else:
                    nc.vector.tensor_scalar(
                        ysb[:], yp[:], -B_SCALE, None, mybir.AluOpType.mult
                    )
                nc.sync.dma_start(out=y_d[:, jp], in_=ysb[:])

    with _ForceCombinedLnExpTables():
        nc.compile()
    return nc


def _get_nc():
    if "nc" not in _CACHE:
        _CACHE["nc"] = _build()
    return _CACHE["nc"]


def kernel(x: np.ndarray, cores: np.ndarray, _trace=False, _trace_kwargs=None):
    x = np.asarray(x, dtype=np.float32)
    cores = np.asarray(cores, dtype=np.float32)

    # x^T in fp16, laid out (p, jp, c, b), c = 2*jj + h
    xt = np.ascontiguousarray(
        x.T.reshape(JP, 4, 128, BATCH).transpose(2, 0, 1, 3).astype(np.float16)
    )
    # cores^T: (i,j,l,k) -> (jp, p, i, h, jj, k), fp16
    wt = (
        cores.transpose(0, 1, 3, 2)
        .reshape(OUT_BLOCKS, JP, 2, 2, 128, NB)
        .transpose(1, 4, 0, 3, 2, 5)
        .astype(np.float16)
    )
    in_maps = [
        {
            "xt": xt,
            "wt": np.ascontiguousarray(
                wt[:, :, c * I_PER_CORE : (c + 1) * I_PER_CORE]
            ),
        }
        for c in range(N_CORES)
    ]

    nc = _get_nc()
    kw = {}
    if _trace:
        kw = dict(trace=True, **(_trace_kwargs or {}))
    out = run_bass_kernel_spmd(nc, in_maps, core_ids=list(range(N_CORES)), **kw)
    if _trace:
        _CACHE["last_result"] = out
    y = np.concatenate(
        [
            r["y"]  # (b, jp, jj, i, k)
            .astype(np.float32)
            .transpose(0, 3, 1, 2, 4)  # (b, i, jp, jj, k)
            .reshape(BATCH, I_PER_CORE, IN_BLOCKS, NB)
            for r in out.results
        ],
        axis=1,
    )
    return y
